# revision 3
# baseline (speedup 1.0000x reference)
"""AttnBlock (GroupNorm -> qkv 1x1 -> softmax attention -> proj -> residual)
for Trainium2, data-parallel over batch across 8 NeuronCores.

Shapes (hardcoded): B=8, C=256, H=W=64, N=H*W=4096, 32 groups.
Each core processes one batch element with channels on SBUF partitions
(C=256 -> 2 partition tiles of 128).

Key layout choices:
  - h, q, k live as [C, N]  (c on partitions)
  - v is computed directly transposed: vT[m, c] = sum_c' h[c', m] wvT[c', c]
  - attention scores are computed transposed: wT[m, n] = sum_o k[o,m] q[o,n]
    so that the second matmul (attn @ v) contracts m on partitions.
  - softmax row-sums via an all-ones [128,128] lhsT matmul which produces the
    sums already broadcast across all 128 partitions.
All heavy matmuls in bf16: the attention path is damped by wp (gain 1e-5),
the numerically-critical residual path (x) is exact fp32.
"""

import numpy as np
import ml_dtypes

import concourse.bass as bass
import concourse.tile as tile
from concourse import bacc, mybir
from concourse.bass_utils import run_bass_kernel_spmd

B, C, H, W = 8, 256, 64, 64
N = H * W            # 4096
G = 32               # num groups
GS = C // G          # 8 channels per group
EPS = 1e-5
P = 128
CT = C // P          # 2 channel tiles
NSB = 8              # n superblocks of 512
SB = N // NSB        # 512
MT = N // P          # 32 m tiles

f32 = mybir.dt.float32
bf16 = mybir.dt.bfloat16
AF = mybir.ActivationFunctionType
ALU = mybir.AluOpType

_CACHE = {}


def _build_program(reps: int = 1):
    nc = bacc.Bacc("TRN2", target_bir_lowering=False, debug=False, num_devices=8)

    x_d = nc.dram_tensor("x", [CT, P, N], f32, kind="ExternalInput")
    wT_d = nc.dram_tensor("wT", [4, CT, P, C], bf16, kind="ExternalInput")
    bq_d = nc.dram_tensor("bq", [P, CT], f32, kind="ExternalInput")
    bk_d = nc.dram_tensor("bk", [P, CT], f32, kind="ExternalInput")
    bp_d = nc.dram_tensor("bp", [P, CT], f32, kind="ExternalInput")
    bv_d = nc.dram_tensor("bv", [1, C], f32, kind="ExternalInput")
    gs_d = nc.dram_tensor("gs", [P, CT], f32, kind="ExternalInput")
    gb_d = nc.dram_tensor("gb", [P, CT], f32, kind="ExternalInput")
    S_d = nc.dram_tensor("S", [CT, P, G], f32, kind="ExternalInput")
    B2_d = nc.dram_tensor("B2", [CT, P, P], f32, kind="ExternalInput")
    out_d = nc.dram_tensor("out", [CT, P, N], f32, kind="ExternalOutput")

    with tile.TileContext(nc) as tc:
        _body(tc, x_d, wT_d, bq_d, bk_d, bp_d, bv_d, gs_d, gb_d, S_d, B2_d,
              out_d, reps)
    nc.finalize()
    return nc


def _body(tc, x_d, wT_d, bq_d, bk_d, bp_d, bv_d, gs_d, gb_d, S_d, B2_d,
          out_d, reps):
    nc = tc.nc

    with (
        tc.tile_pool(name="const", bufs=1) as const,
        tc.tile_pool(name="big", bufs=1) as big,
        tc.tile_pool(name="ew", bufs=2) as ewp,
        tc.tile_pool(name="small", bufs=2) as small,
        tc.tile_pool(name="pmm", bufs=3, space="PSUM") as pmm,
        tc.tile_pool(name="pht", bufs=2, space="PSUM") as pht,
        tc.tile_pool(name="prs", bufs=1, space="PSUM") as prsp,
        tc.tile_pool(name="pout", bufs=2, space="PSUM") as pout,
    ):
        # ---- constant loads (once) ----
        wT_sb = const.tile([P, 4, CT, C], bf16)
        nc.sync.dma_start(out=wT_sb, in_=wT_d.ap().rearrange("w k p o -> p w k o"))
        bq_sb = const.tile([P, CT], f32)
        nc.sync.dma_start(out=bq_sb, in_=bq_d.ap())
        bk_sb = const.tile([P, CT], f32)
        nc.sync.dma_start(out=bk_sb, in_=bk_d.ap())
        bp_sb = const.tile([P, CT], f32)
        nc.sync.dma_start(out=bp_sb, in_=bp_d.ap())
        gs_sb = const.tile([P, CT], f32)
        nc.sync.dma_start(out=gs_sb, in_=gs_d.ap())
        gb_sb = const.tile([P, CT], f32)
        nc.sync.dma_start(out=gb_sb, in_=gb_d.ap())
        S_sb = const.tile([P, CT, G], f32)
        nc.sync.dma_start(out=S_sb, in_=S_d.ap().rearrange("k p g -> p k g"))
        B2_sb = const.tile([P, CT, P], f32)
        nc.sync.dma_start(out=B2_sb, in_=B2_d.ap().rearrange("k p c -> p k c"))
        # bv broadcast to all partitions
        bv_sb = const.tile([P, C], f32)
        bv_bcast = bass.AP(tensor=bv_d.ap().tensor, offset=0,
                           ap=[[0, P], [1, C]])
        nc.sync.dma_start(out=bv_sb, in_=bv_bcast)
        ones_bf = const.tile([P, P], bf16)
        nc.vector.memset(ones_bf, 1.0)
        eps_sb = const.tile([P, 1], f32)
        nc.vector.memset(eps_sb, EPS)

        for _ in range(reps):
            # ---- load x ----
            x_sb = big.tile([P, CT, N], f32, tag="x")
            nc.sync.dma_start(out=x_sb, in_=x_d.ap().rearrange("t p n -> p t n"))

            # ---- GroupNorm stats ----
            # per-channel mean/var via bn_stats/bn_aggr
            stats_in = small.tile([P, CT, 2], f32, tag="stats_in")
            for cb in range(CT):
                bnst = small.tile([P, 8, 6], f32, tag="bnst")
                xg = x_sb[:, cb, :].rearrange("p (s f) -> p s f", f=512)
                for s in range(8):
                    nc.vector.bn_stats(out=bnst[:, s, :], in_=xg[:, s, :])
                mv = small.tile([P, 2], f32, tag="mv")
                nc.vector.bn_aggr(out=mv, in_=bnst)
                # stats_in[:, cb, 0] = mean ; stats_in[:, cb, 1] = var + mean^2
                sq = small.tile([P, 1], f32, tag="sq")
                nc.vector.tensor_mul(sq, mv[:, 0:1], mv[:, 0:1])
                nc.vector.tensor_add(stats_in[:, cb, 1:2], mv[:, 1:2], sq)
                nc.vector.tensor_copy(stats_in[:, cb, 0:1], mv[:, 0:1])

            # group reduce: [32, 2] = sum over channels in group
            pg = pmm.tile([G, 2], f32, tag="pw")
            for cb in range(CT):
                nc.tensor.matmul(pg, S_sb[:, cb, :], stats_in[:, cb, :],
                                 start=(cb == 0), stop=(cb == CT - 1))
            gstats = small.tile([P, 2], f32, tag="gstats")
            nc.vector.memset(gstats, 0.0)
            # gstats[:, 0] = group mean, gstats[:, 1] = group E[x^2]
            nc.vector.tensor_scalar_mul(gstats[:G, :], pg, 1.0 / GS)
            gvar = small.tile([P, 1], f32, tag="gvar")
            nc.vector.tensor_mul(gvar[:G], gstats[:G, 0:1], gstats[:G, 0:1])
            nc.vector.tensor_sub(gvar[:G], gstats[:G, 1:2], gvar[:G])
            # 1/sqrt(var+eps)
            nc.scalar.activation(out=gvar[:G], in_=gvar[:G], func=AF.Sqrt,
                                 bias=eps_sb[:G], scale=1.0)
            nc.vector.reciprocal(gstats[:G, 1:2], gvar[:G])

            # broadcast to channels + apply
            h_sb = big.tile([P, CT, N], bf16, tag="ew")  # shares slots with expw
            for cb in range(CT):
                pmi = pmm.tile([P, 2], f32, tag="pw")
                nc.tensor.matmul(pmi, B2_sb[:, cb, :], gstats,
                                 start=True, stop=True)
                ab = small.tile([P, 2], f32, tag="ab")
                # a = inv * gn_scale ; b = gn_bias - mean * a
                nc.vector.tensor_mul(ab[:, 0:1], pmi[:, 1:2], gs_sb[:, cb:cb + 1])
                tmp = small.tile([P, 1], f32, tag="tmp")
                nc.vector.tensor_mul(tmp, pmi[:, 0:1], ab[:, 0:1])
                nc.vector.tensor_sub(ab[:, 1:2], gb_sb[:, cb:cb + 1], tmp)
                nc.vector.tensor_scalar(out=h_sb[:, cb, :], in0=x_sb[:, cb, :],
                                        scalar1=ab[:, 0:1], scalar2=ab[:, 1:2],
                                        op0=ALU.mult, op1=ALU.add)

            # ---- q, k projections: q[o, n] = sum_c' wqT[c', o] h[c', n] ----
            q_sb = big.tile([P, CT, N], bf16, tag="q")
            k_sb = big.tile([P, CT, N], bf16, tag="k")
            for wsel, dst, bias in ((0, q_sb, bq_sb), (1, k_sb, bk_sb)):
                for ob in range(CT):
                    for ch in range(NSB):
                        pt = pmm.tile([P, SB], f32, tag="pw")
                        for kt in range(CT):
                            nc.tensor.matmul(
                                pt,
                                wT_sb[:, wsel, kt, ob * P:(ob + 1) * P],
                                h_sb[:, kt, ch * SB:(ch + 1) * SB],
                                start=(kt == 0), stop=(kt == CT - 1))
                        nc.vector.tensor_scalar(
                            out=dst[:, ob, ch * SB:(ch + 1) * SB], in0=pt,
                            scalar1=bias[:, ob:ob + 1], scalar2=None,
                            op0=ALU.add)

            # ---- vT[m, c] = sum_c' h[c', m] wvT[c', c]  (+ bv) ----
            vT_sb = big.tile([P, MT, C], bf16, tag="vT")
            for mt in range(MT):
                pt = pmm.tile([P, C], f32, tag="pw")
                for kt in range(CT):
                    nc.tensor.matmul(pt, h_sb[:, kt, mt * P:(mt + 1) * P],
                                     wT_sb[:, 2, kt, :],
                                     start=(kt == 0), stop=(kt == CT - 1))
                nc.vector.tensor_add(vT_sb[:, mt, :], pt, bv_sb)

            # ---- attention over n-superblocks ----
            for sb in range(NSB):
                nsl = slice(sb * SB, (sb + 1) * SB)
                # scores^T block: wT[m, n] for m in [0,4096), n in superblock
                ew = ewp.tile([P, MT, SB], bf16, tag="ew")
                for mt in range(MT):
                    pw = pmm.tile([P, SB], f32, tag="pw")
                    for kt in range(CT):
                        nc.tensor.matmul(pw, k_sb[:, kt, mt * P:(mt + 1) * P],
                                         q_sb[:, kt, nsl],
                                         start=(kt == 0), stop=(kt == CT - 1))
                    # exp(scores / sqrt(C)), fused with PSUM->SBUF copy
                    nc.scalar.activation(out=ew[:, mt, :], in_=pw, func=AF.Exp,
                                         scale=C ** -0.5)

                # hattT[c, n] = sum_m vT[m, c] expwT[m, n]; rowsum broadcast
                ph0 = pht.tile([P, SB], f32, tag="ph")
                ph1 = pht.tile([P, SB], f32, tag="ph")
                prs = prsp.tile([P, SB], f32, tag="prs")
                for mt in range(MT):
                    st, sp = (mt == 0), (mt == MT - 1)
                    nc.tensor.matmul(ph0, vT_sb[:, mt, 0:P], ew[:, mt, :],
                                     start=st, stop=sp)
                    nc.tensor.matmul(ph1, vT_sb[:, mt, P:C], ew[:, mt, :],
                                     start=st, stop=sp)
                    nc.tensor.matmul(prs, ones_bf, ew[:, mt, :],
                                     start=st, stop=sp)

                recip = small.tile([P, SB], f32, tag="recip")
                nc.vector.reciprocal(recip, prs)
                hatt = small.tile([P, CT, SB], bf16, tag="hatt")
                nc.vector.tensor_mul(hatt[:, 0, :], ph0, recip)
                nc.vector.tensor_mul(hatt[:, 1, :], ph1, recip)

                # proj + bias + residual
                out_t = small.tile([P, CT, SB], f32, tag="out")
                for ob in range(CT):
                    po = pout.tile([P, SB], f32, tag="po")
                    for cb in range(CT):
                        nc.tensor.matmul(po, wT_sb[:, 3, cb, ob * P:(ob + 1) * P],
                                         hatt[:, cb, :],
                                         start=(cb == 0), stop=(cb == CT - 1))
                    nc.vector.tensor_scalar(out=out_t[:, ob, :], in0=po,
                                            scalar1=bp_sb[:, ob:ob + 1],
                                            scalar2=None, op0=ALU.add)
                    nc.vector.tensor_add(out_t[:, ob, :], out_t[:, ob, :],
                                         x_sb[:, ob, nsl])
                    nc.sync.dma_start(out=out_d.ap()[ob, :, nsl],
                                      in_=out_t[:, ob, :])


def _get_program(reps: int = 1):
    key = ("prog", reps)
    if key not in _CACHE:
        _CACHE[key] = _build_program(reps)
    return _CACHE[key]


def _host_params(gn_scale, gn_bias, wq, bq, wk, bk, wv, bv, wp, bp):
    def percol(v):  # [C] -> [128, CT] with v[t*128+p] at [p, t]
        return np.ascontiguousarray(v.reshape(CT, P).T.astype(np.float32))

    wT = np.stack([
        np.ascontiguousarray(w.T).reshape(CT, P, C)
        for w in (wq, wk, wv, wp)
    ]).astype(ml_dtypes.bfloat16)

    p_idx = np.arange(P)
    S = np.zeros((CT, P, G), np.float32)
    B2 = np.zeros((CT, P, P), np.float32)
    for cb in range(CT):
        grp = (cb * P + p_idx) // GS          # group id of channel cb*128+p
        S[cb, p_idx, grp] = 1.0
        B2[cb, grp, p_idx] = 1.0              # [g, c] selector
    return {
        "wT": wT,
        "bq": percol(bq), "bk": percol(bk), "bp": percol(bp),
        "bv": np.ascontiguousarray(bv.reshape(1, C).astype(np.float32)),
        "gs": percol(gn_scale), "gb": percol(gn_bias),
        "S": S, "B2": B2,
    }


def kernel(x, gn_scale, gn_bias, wq, bq, wk, bk, wv, bv, wp, bp):
    x = np.asarray(x, np.float32)
    params = _host_params(*(np.asarray(a) for a in (
        gn_scale, gn_bias, wq, bq, wk, bk, wv, bv, wp, bp)))
    nc = _get_program()
    in_maps = [
        {"x": np.ascontiguousarray(x[b].reshape(CT, P, N)), **params}
        for b in range(B)
    ]
    res = run_bass_kernel_spmd(nc, in_maps, core_ids=list(range(B)))
    out = np.stack([r["out"] for r in res.results])  # [B, CT, P, N]
    return out.reshape(B, C, H, W).astype(np.float32)


if __name__ == "__main__":
    rng = np.random.default_rng(0)
    x = rng.standard_normal((B, C, H, W), dtype=np.float32)
    ins = dict(
        x=x,
        gn_scale=np.ones(C, np.float32), gn_bias=np.zeros(C, np.float32),
        wq=rng.standard_normal((C, C), dtype=np.float32) * 0.05,
        bq=np.zeros(C, np.float32),
        wk=rng.standard_normal((C, C), dtype=np.float32) * 0.05,
        bk=np.zeros(C, np.float32),
        wv=rng.standard_normal((C, C), dtype=np.float32) * 0.05,
        bv=np.zeros(C, np.float32),
        wp=rng.standard_normal((C, C), dtype=np.float32) * 1e-5,
        bp=np.zeros(C, np.float32),
    )
    out = kernel(**ins)
    print("out", out.shape, out.dtype, np.abs(out).max())


# revision 11
# speedup vs baseline: 217.6533x; 217.6533x over previous
"""AttnBlock (GroupNorm -> qkv 1x1 -> softmax attention -> proj -> residual)
for Trainium2, data-parallel over batch across 8 NeuronCores.

Shapes (hardcoded): B=8, C=256, H=W=64, N=H*W=4096, 32 groups.
Each core processes one batch element with channels on SBUF partitions
(C=256 -> 2 partition tiles of 128).

Key layout choices:
  - h, q, k live as [C, N]  (c on partitions)
  - v is computed directly transposed: vT[m, c] = sum_c' h[c', m] wvT[c', c]
  - attention scores are computed transposed: wT[m, n] = sum_o k[o,m] q[o,n]
    so that the second matmul (attn @ v) contracts m on partitions.
  - softmax row-sums via an all-ones [128,128] lhsT matmul which produces the
    sums already broadcast across all 128 partitions.
All heavy matmuls in bf16: the attention path is damped by wp (gain 1e-5),
the numerically-critical residual path (x) is exact fp32.
"""

import numpy as np
import ml_dtypes

import concourse.bass as bass
import concourse.tile as tile
from concourse import bacc, mybir

B, C, H, W = 8, 256, 64, 64
N = H * W            # 4096
G = 32               # num groups
GS = C // G          # 8 channels per group
EPS = 1e-5
P = 128
CT = C // P          # 2 channel tiles
NSB = 8              # n superblocks of 512
SB = N // NSB        # 512
MT = N // P          # 32 m tiles

f32 = mybir.dt.float32
bf16 = mybir.dt.bfloat16
AF = mybir.ActivationFunctionType
ALU = mybir.AluOpType

_CACHE = {}


def _build_program(reps: int = 1, loop_n: int = 1):
    nc = bacc.Bacc("TRN2", target_bir_lowering=False, debug=False, num_devices=8)

    x_d = nc.dram_tensor("x", [CT, P, N], f32, kind="ExternalInput")
    wT_d = nc.dram_tensor("wT", [4, CT, P, C], bf16, kind="ExternalInput")
    bq_d = nc.dram_tensor("bq", [P, CT], f32, kind="ExternalInput")
    bk_d = nc.dram_tensor("bk", [P, CT], f32, kind="ExternalInput")
    bp_d = nc.dram_tensor("bp", [P, CT], f32, kind="ExternalInput")
    bv_d = nc.dram_tensor("bv", [1, C], f32, kind="ExternalInput")
    gs_d = nc.dram_tensor("gs", [P, CT], f32, kind="ExternalInput")
    gb_d = nc.dram_tensor("gb", [P, CT], f32, kind="ExternalInput")
    S_d = nc.dram_tensor("S", [CT, P, G], f32, kind="ExternalInput")
    B2_d = nc.dram_tensor("B2", [CT, P, P], f32, kind="ExternalInput")
    out_d = nc.dram_tensor("out", [CT, P, N], f32, kind="ExternalOutput")

    with tile.TileContext(nc) as tc:
        _body(tc, x_d, wT_d, bq_d, bk_d, bp_d, bv_d, gs_d, gb_d, S_d, B2_d,
              out_d, reps, loop_n)
    nc.finalize()
    return nc


def _body(tc, x_d, wT_d, bq_d, bk_d, bp_d, bv_d, gs_d, gb_d, S_d, B2_d,
          out_d, reps, loop_n=1):
    nc = tc.nc

    with (
        tc.tile_pool(name="const", bufs=1) as const,
        tc.tile_pool(name="big", bufs=1) as big,
        tc.tile_pool(name="ew", bufs=2) as ewp,
        tc.tile_pool(name="small", bufs=2) as small,
        tc.tile_pool(name="pmm", bufs=3, space="PSUM") as pmm,
        tc.tile_pool(name="pht", bufs=2, space="PSUM") as pht,
        tc.tile_pool(name="prs", bufs=1, space="PSUM") as prsp,
        tc.tile_pool(name="pout", bufs=2, space="PSUM") as pout,
    ):
        # ---- constant loads (once) ----
        wT_sb = const.tile([P, 4, CT, C], bf16)
        nc.sync.dma_start(out=wT_sb, in_=wT_d.ap().rearrange("w k p o -> p w k o"))
        bq_sb = const.tile([P, CT], f32)
        nc.sync.dma_start(out=bq_sb, in_=bq_d.ap())
        bk_sb = const.tile([P, CT], f32)
        nc.sync.dma_start(out=bk_sb, in_=bk_d.ap())
        bp_sb = const.tile([P, CT], f32)
        nc.sync.dma_start(out=bp_sb, in_=bp_d.ap())
        gs_sb = const.tile([P, CT], f32)
        nc.sync.dma_start(out=gs_sb, in_=gs_d.ap())
        gb_sb = const.tile([P, CT], f32)
        nc.sync.dma_start(out=gb_sb, in_=gb_d.ap())
        S_sb = const.tile([P, CT, G], f32)
        nc.sync.dma_start(out=S_sb, in_=S_d.ap().rearrange("k p g -> p k g"))
        B2_sb = const.tile([P, CT, P], f32)
        nc.sync.dma_start(out=B2_sb, in_=B2_d.ap().rearrange("k p c -> p k c"))
        # bv broadcast to all partitions
        bv_sb = const.tile([P, C], f32)
        bv_bcast = bass.AP(tensor=bv_d.ap().tensor, offset=0,
                           ap=[[0, P], [1, C]])
        nc.sync.dma_start(out=bv_sb, in_=bv_bcast)
        ones_bf = const.tile([P, P], bf16)
        nc.vector.memset(ones_bf, 1.0)
        eps_sb = const.tile([P, 1], f32)
        nc.vector.memset(eps_sb, EPS)

        def one_iter():
            # ---- load x ----
            x_sb = big.tile([P, CT, N], f32, tag="x")
            nc.sync.dma_start(out=x_sb, in_=x_d.ap().rearrange("t p n -> p t n"))

            # ---- GroupNorm stats ----
            # per-channel mean/var via bn_stats/bn_aggr
            stats_in = small.tile([P, CT, 2], f32, tag="stats_in")
            for cb in range(CT):
                bnst = small.tile([P, 8, 6], f32, tag="bnst")
                xg = x_sb[:, cb, :].rearrange("p (s f) -> p s f", f=512)
                for s in range(8):
                    nc.vector.bn_stats(out=bnst[:, s, :], in_=xg[:, s, :])
                mv = small.tile([P, 2], f32, tag="mv")
                nc.vector.bn_aggr(out=mv, in_=bnst)
                # stats_in[:, cb, 0] = mean ; stats_in[:, cb, 1] = var + mean^2
                sq = small.tile([P, 1], f32, tag="sq")
                nc.vector.tensor_mul(sq, mv[:, 0:1], mv[:, 0:1])
                nc.vector.tensor_add(stats_in[:, cb, 1:2], mv[:, 1:2], sq)
                nc.vector.tensor_copy(stats_in[:, cb, 0:1], mv[:, 0:1])

            # group reduce: [32, 2] = sum over channels in group
            pg = pmm.tile([G, 2], f32, tag="pw")
            for cb in range(CT):
                nc.tensor.matmul(pg, S_sb[:, cb, :], stats_in[:, cb, :],
                                 start=(cb == 0), stop=(cb == CT - 1))
            gstats = small.tile([P, 2], f32, tag="gstats")
            nc.vector.memset(gstats, 0.0)
            # gstats[:, 0] = group mean, gstats[:, 1] = group E[x^2]
            nc.vector.tensor_scalar_mul(gstats[:G, :], pg, 1.0 / GS)
            gvar = small.tile([P, 1], f32, tag="gvar")
            nc.vector.tensor_mul(gvar[:G], gstats[:G, 0:1], gstats[:G, 0:1])
            nc.vector.tensor_sub(gvar[:G], gstats[:G, 1:2], gvar[:G])
            # 1/sqrt(var+eps)
            nc.scalar.activation(out=gvar[:G], in_=gvar[:G], func=AF.Sqrt,
                                 bias=eps_sb[:G], scale=1.0)
            nc.vector.reciprocal(gstats[:G, 1:2], gvar[:G])

            # broadcast to channels + apply
            h_sb = big.tile([P, CT, N], bf16, tag="ew")  # shares slots with expw
            for cb in range(CT):
                pmi = pmm.tile([P, 2], f32, tag="pw")
                nc.tensor.matmul(pmi, B2_sb[:, cb, :], gstats,
                                 start=True, stop=True)
                ab = small.tile([P, 2], f32, tag="ab")
                # a = inv * gn_scale ; b = gn_bias - mean * a
                nc.vector.tensor_mul(ab[:, 0:1], pmi[:, 1:2], gs_sb[:, cb:cb + 1])
                tmp = small.tile([P, 1], f32, tag="tmp")
                nc.vector.tensor_mul(tmp, pmi[:, 0:1], ab[:, 0:1])
                nc.vector.tensor_sub(ab[:, 1:2], gb_sb[:, cb:cb + 1], tmp)
                nc.vector.tensor_scalar(out=h_sb[:, cb, :], in0=x_sb[:, cb, :],
                                        scalar1=ab[:, 0:1], scalar2=ab[:, 1:2],
                                        op0=ALU.mult, op1=ALU.add)

            # ---- q, k projections: q[o, n] = sum_c' wqT[c', o] h[c', n] ----
            q_sb = big.tile([P, CT, N], bf16, tag="q")
            k_sb = big.tile([P, CT, N], bf16, tag="k")
            for wsel, dst, bias in ((0, q_sb, bq_sb), (1, k_sb, bk_sb)):
                for ob in range(CT):
                    for ch in range(NSB):
                        pt = pmm.tile([P, SB], f32, tag="pw")
                        for kt in range(CT):
                            nc.tensor.matmul(
                                pt,
                                wT_sb[:, wsel, kt, ob * P:(ob + 1) * P],
                                h_sb[:, kt, ch * SB:(ch + 1) * SB],
                                start=(kt == 0), stop=(kt == CT - 1))
                        nc.vector.tensor_scalar(
                            out=dst[:, ob, ch * SB:(ch + 1) * SB], in0=pt,
                            scalar1=bias[:, ob:ob + 1], scalar2=None,
                            op0=ALU.add)

            # ---- vT[m, c] = sum_c' h[c', m] wvT[c', c]  (+ bv) ----
            vT_sb = big.tile([P, MT, C], bf16, tag="vT")
            for mt in range(MT):
                pt = pmm.tile([P, C], f32, tag="pw")
                for kt in range(CT):
                    nc.tensor.matmul(pt, h_sb[:, kt, mt * P:(mt + 1) * P],
                                     wT_sb[:, 2, kt, :],
                                     start=(kt == 0), stop=(kt == CT - 1))
                nc.vector.tensor_add(vT_sb[:, mt, :], pt, bv_sb)

            # ---- attention over n-superblocks ----
            for sb in range(NSB):
                nsl = slice(sb * SB, (sb + 1) * SB)
                # scores^T block: wT[m, n] for m in [0,4096), n in superblock
                ew = ewp.tile([P, MT, SB], bf16, tag="ew")
                for mt in range(MT):
                    pw = pmm.tile([P, SB], f32, tag="pw")
                    for kt in range(CT):
                        nc.tensor.matmul(pw, k_sb[:, kt, mt * P:(mt + 1) * P],
                                         q_sb[:, kt, nsl],
                                         start=(kt == 0), stop=(kt == CT - 1))
                    # exp(scores / sqrt(C)), fused with PSUM->SBUF copy
                    nc.scalar.activation(out=ew[:, mt, :], in_=pw, func=AF.Exp,
                                         scale=C ** -0.5)

                # hattT[c, n] = sum_m vT[m, c] expwT[m, n]; rowsum broadcast
                ph0 = pht.tile([P, SB], f32, tag="ph")
                ph1 = pht.tile([P, SB], f32, tag="ph")
                prs = prsp.tile([P, SB], f32, tag="prs")
                for mt in range(MT):
                    st, sp = (mt == 0), (mt == MT - 1)
                    nc.tensor.matmul(ph0, vT_sb[:, mt, 0:P], ew[:, mt, :],
                                     start=st, stop=sp)
                    nc.tensor.matmul(ph1, vT_sb[:, mt, P:C], ew[:, mt, :],
                                     start=st, stop=sp)
                    nc.tensor.matmul(prs, ones_bf, ew[:, mt, :],
                                     start=st, stop=sp)

                recip = small.tile([P, SB], f32, tag="recip")
                nc.vector.reciprocal(recip, prs)
                hatt = small.tile([P, CT, SB], bf16, tag="hatt")
                nc.vector.tensor_mul(hatt[:, 0, :], ph0, recip)
                nc.vector.tensor_mul(hatt[:, 1, :], ph1, recip)

                # proj + bias + residual
                out_t = small.tile([P, CT, SB], f32, tag="out")
                for ob in range(CT):
                    po = pout.tile([P, SB], f32, tag="po")
                    for cb in range(CT):
                        nc.tensor.matmul(po, wT_sb[:, 3, cb, ob * P:(ob + 1) * P],
                                         hatt[:, cb, :],
                                         start=(cb == 0), stop=(cb == CT - 1))
                    nc.vector.tensor_scalar(out=out_t[:, ob, :], in0=po,
                                            scalar1=bp_sb[:, ob:ob + 1],
                                            scalar2=None, op0=ALU.add)
                    nc.vector.tensor_add(out_t[:, ob, :], out_t[:, ob, :],
                                         x_sb[:, ob, nsl])
                    nc.sync.dma_start(out=out_d.ap()[ob, :, nsl],
                                      in_=out_t[:, ob, :])

        for _ in range(reps):
            if loop_n > 1:
                with tc.For_i(0, loop_n, 1):
                    one_iter()
            else:
                one_iter()


def _get_program(reps: int = 1, loop_n: int = 1):
    key = ("prog", reps, loop_n)
    if key not in _CACHE:
        _CACHE[key] = _build_program(reps, loop_n)
    return _CACHE[key]


def _make_runner(nc, n_cores):
    """Like bass2jax.run_bass_via_pjrt, but the jitted callable is built once
    and reused -- run_bass_via_pjrt re-jits (and thus recompiles) per call."""
    import jax
    from jax.sharding import Mesh, PartitionSpec
    from jax.experimental.shard_map import shard_map
    from concourse import bass2jax

    bass2jax.install_neuronx_cc_hook()
    in_names, out_names, out_avals, zero_shapes = [], [], [], []
    pname = nc.partition_id_tensor.name if nc.partition_id_tensor else None
    for alloc in nc.m.functions[0].allocations:
        if not isinstance(alloc, mybir.MemoryLocationSet):
            continue
        name = alloc.memorylocations[0].name
        if alloc.kind == "ExternalInput":
            if name != pname:
                in_names.append(name)
        elif alloc.kind == "ExternalOutput":
            out_names.append(name)
            shape, dtype = tuple(alloc.tensor_shape), mybir.dt.np(alloc.dtype)
            out_avals.append(jax.core.ShapedArray(shape, dtype))
            zero_shapes.append((shape, dtype))
    n_params, n_outs = len(in_names), len(out_avals)
    all_in = in_names + out_names + ([pname] if pname else [])

    def _bd(*args):
        operands = list(args)
        if pname is not None:
            operands.append(bass2jax.partition_id_tensor())
        outs = bass2jax._bass_exec_p.bind(
            *operands, out_avals=tuple(out_avals),
            in_names=tuple(all_in), out_names=tuple(out_names),
            lowering_input_output_aliases=(), sim_require_finite=True,
            sim_require_nnan=True, nc=nc)
        return tuple(outs)

    donate = tuple(range(n_params, n_params + n_outs))
    devices = jax.devices()[:n_cores]
    mesh = Mesh(np.asarray(devices), ("core",))
    in_specs = (PartitionSpec("core"),) * (n_params + n_outs)
    out_specs = (PartitionSpec("core"),) * n_outs
    sharded = jax.jit(shard_map(_bd, mesh=mesh, in_specs=in_specs,
                                out_specs=out_specs, check_rep=False),
                      donate_argnums=donate, keep_unused=True)

    def run(in_maps):
        per_core = [[np.asarray(m[name]) for name in in_names] for m in in_maps]
        concat_in = [np.concatenate([per_core[c][i] for c in range(n_cores)], 0)
                     for i in range(n_params)]
        concat_zeros = [np.zeros((n_cores * s[0], *s[1:]), d)
                        for (s, d) in zero_shapes]
        out_arrs = sharded(*concat_in, *concat_zeros)
        jax.block_until_ready(out_arrs)
        return [
            {name: np.asarray(out_arrs[i]).reshape(n_cores, *out_avals[i].shape)[c]
             for i, name in enumerate(out_names)}
            for c in range(n_cores)
        ]
    return run


def _get_runner(reps: int = 1, loop_n: int = 1):
    key = ("runner", reps, loop_n)
    if key not in _CACHE:
        _CACHE[key] = _make_runner(_get_program(reps, loop_n), B)
    return _CACHE[key]


def _host_params(gn_scale, gn_bias, wq, bq, wk, bk, wv, bv, wp, bp):
    def percol(v):  # [C] -> [128, CT] with v[t*128+p] at [p, t]
        return np.ascontiguousarray(v.reshape(CT, P).T.astype(np.float32))

    wT = np.stack([
        np.ascontiguousarray(w.T).reshape(CT, P, C)
        for w in (wq, wk, wv, wp)
    ]).astype(ml_dtypes.bfloat16)

    p_idx = np.arange(P)
    S = np.zeros((CT, P, G), np.float32)
    B2 = np.zeros((CT, P, P), np.float32)
    for cb in range(CT):
        grp = (cb * P + p_idx) // GS          # group id of channel cb*128+p
        S[cb, p_idx, grp] = 1.0
        B2[cb, grp, p_idx] = 1.0              # [g, c] selector
    return {
        "wT": wT,
        "bq": percol(bq), "bk": percol(bk), "bp": percol(bp),
        "bv": np.ascontiguousarray(bv.reshape(1, C).astype(np.float32)),
        "gs": percol(gn_scale), "gb": percol(gn_bias),
        "S": S, "B2": B2,
    }


def kernel(x, gn_scale, gn_bias, wq, bq, wk, bk, wv, bv, wp, bp):
    x = np.asarray(x, np.float32)
    params = _host_params(*(np.asarray(a) for a in (
        gn_scale, gn_bias, wq, bq, wk, bk, wv, bv, wp, bp)))
    run = _get_runner()
    in_maps = [
        {"x": np.ascontiguousarray(x[b].reshape(CT, P, N)), **params}
        for b in range(B)
    ]
    res = run(in_maps)
    out = np.stack([r["out"] for r in res])  # [B, CT, P, N]
    return out.reshape(B, C, H, W).astype(np.float32)


if __name__ == "__main__":
    rng = np.random.default_rng(0)
    x = rng.standard_normal((B, C, H, W), dtype=np.float32)
    ins = dict(
        x=x,
        gn_scale=np.ones(C, np.float32), gn_bias=np.zeros(C, np.float32),
        wq=rng.standard_normal((C, C), dtype=np.float32) * 0.05,
        bq=np.zeros(C, np.float32),
        wk=rng.standard_normal((C, C), dtype=np.float32) * 0.05,
        bk=np.zeros(C, np.float32),
        wv=rng.standard_normal((C, C), dtype=np.float32) * 0.05,
        bv=np.zeros(C, np.float32),
        wp=rng.standard_normal((C, C), dtype=np.float32) * 1e-5,
        bp=np.zeros(C, np.float32),
    )
    out = kernel(**ins)
    print("out", out.shape, out.dtype, np.abs(out).max())
